# revision 15
# baseline (speedup 1.0000x reference)
"""Trainium2 Bass kernel for the Elman-RNN reference (nn_Baseline_78005196030499).

Architecture (per core, data-parallel over batch, B=128 -> 16 rows/core):
  1. Host: cast emb to fp16, pre-transpose weights, build t-major int32 token
     index tables.
  2. Device prep: indirect-DMA gather of embedding rows (fp16), PE transpose
     to put E on partitions, fp16 input-projection GEMM (fp32 PSUM) with
     Wb+Ub folded in via the ACT epilogue -> wx [128, T*64] fp32 resident in
     SBUF (layout col = t*64 + jb*16 + b).
  3. Device scan: 512 sequential steps; per step 16 fp16 matmuls
     (stationary UwT tiles [128,128], moving hT [128,16]) accumulating into
     4 PSUM banks (one per output j-block), drained by DVE add (psum + wx_t)
     and ACT tanh back to fp16 hT tiles.  j-major order lets the drains trail
     the PE stream so PE never stalls.
  4. Final hidden state written out in fp32; the tiny V-projection and the
     batch gather happen on host in fp32.
"""
import sys

sys.path.insert(0, "/opt/trn_rl_repo")

import numpy as np

import concourse.bass as bass
import concourse.tile as tile
from concourse import bacc, mybir
from concourse.masks import make_identity

# Problem shape (hardcoded per contract).
VOCAB, EMB, HID, OUT = 50257, 256, 512, 2
B, T = 128, 512
NCORES = 8
BL = B // NCORES          # batch rows per core = 16
NTOK = BL * T             # tokens per core = 8192
P = 128
NJ = HID // P             # 4 j-blocks
NK = HID // P             # 4 k-chunks
NE = EMB // P             # 2 e-chunks
CHUNK = 512               # tokens per GEMM sub-block
GCHUNK = 512              # tokens per gather chunk (HW limit: ~512 idxs/gather)
NGC = NTOK // GCHUNK      # 4
GT = GCHUNK // BL         # timesteps per gather chunk = 128
VLO = 32767               # tokens < VLO come from emb_lo (row tok+1)
NLO = VLO + 1             # emb_lo rows (incl. leading zero row)
NHI = VOCAB - VLO + 1     # emb_hi rows (incl. leading zero row)

F16 = mybir.dt.float16
F32 = mybir.dt.float32
I16 = mybir.dt.int16

_CACHED = {}


def build_module():
    nc = bacc.Bacc("TRN2", target_bir_lowering=False, debug=False)

    emb_lo_d = nc.dram_tensor("emb_lo16", [NLO, EMB], F16, kind="ExternalInput")
    emb_hi_d = nc.dram_tensor("emb_hi16", [NHI, EMB], F16, kind="ExternalInput")
    wwT_d = nc.dram_tensor("wwT16", [P, NE, HID], F16, kind="ExternalInput")
    uwT_d = nc.dram_tensor("uwT16", [P, NK, HID], F16, kind="ExternalInput")
    bias_d = nc.dram_tensor("bias32", [P, NJ], F32, kind="ExternalInput")
    idxlo_d = nc.dram_tensor("idx_lo16", [P, NTOK // 16], I16, kind="ExternalInput")
    idxhi_d = nc.dram_tensor("idx_hi16", [P, NTOK // 16], I16, kind="ExternalInput")
    out_d = nc.dram_tensor("ht_out", [P, NJ * BL], F32, kind="ExternalOutput")

    with tile.TileContext(nc) as tc:
        with (
            tc.tile_pool(name="const", bufs=1) as cpool,
            tc.tile_pool(name="wxpool", bufs=1) as wxpool,
            tc.tile_pool(name="gather", bufs=2) as gpool,
            tc.tile_pool(name="scan_sb", bufs=8) as spool,
        ):
            # --- resident constants -------------------------------------
            uwT = cpool.tile([P, NK, HID], F16)
            nc.sync.dma_start(uwT[:], uwT_d[:])
            wwT = cpool.tile([P, NE, HID], F16)
            nc.sync.dma_start(wwT[:], wwT_d[:])
            bias = cpool.tile([P, NJ], F32)
            nc.sync.dma_start(bias[:], bias_d[:])
            idxlo = cpool.tile([P, NTOK // 16], I16)
            nc.sync.dma_start(idxlo[:], idxlo_d[:])
            idxhi = cpool.tile([P, NTOK // 16], I16)
            nc.sync.dma_start(idxhi[:], idxhi_d[:])
            ident = cpool.tile([P, P], F16)
            make_identity(nc, ident[:])

            # wx buffer: [128, T * 64] fp16, col = t*64 + jb*16 + b
            wx = wxpool.tile([P, T * NJ * BL], F16)
            wx_view = wx[:].rearrange("p (t j b) -> p t j b", t=T, j=NJ, b=BL)

            # --- prep: transposing gather + GEMM ------------------------
            # dma_gather(transpose=True) lands rows directly in the
            # [e-partition, e-chunk, token] layout the GEMM needs.  The
            # vocab is split across two fp16 tables (int16 index limit),
            # each with a leading all-zero row; out-of-range tokens hit the
            # zero row, so the two gathers simply sum.
            with tc.tile_pool(name="prep_ps", bufs=2, space="PSUM") as ppool:
                for c in range(NGC):
                    isl = slice(c * (GCHUNK // 16), (c + 1) * (GCHUNK // 16))
                    g_lo = gpool.tile([P, NE, GCHUNK], F16)
                    nc.gpsimd.dma_gather(
                        out_ap=g_lo[:], in_ap=emb_lo_d[:],
                        idxs_ap=idxlo[:, isl],
                        num_idxs=GCHUNK, num_idxs_reg=GCHUNK,
                        elem_size=EMB, transpose=True,
                    )
                    g_hi = gpool.tile([P, NE, GCHUNK], F16)
                    nc.gpsimd.dma_gather(
                        out_ap=g_hi[:], in_ap=emb_hi_d[:],
                        idxs_ap=idxhi[:, isl],
                        num_idxs=GCHUNK, num_idxs_reg=GCHUNK,
                        elem_size=EMB, transpose=True,
                    )
                    nc.vector.tensor_add(g_lo[:], g_lo[:], g_hi[:])
                    for s in range(GCHUNK // CHUNK):
                        ssl = slice(s * CHUNK, (s + 1) * CHUNK)
                        for jb in range(NJ):
                            wxps = ppool.tile([P, CHUNK], F32)
                            for e in range(NE):
                                nc.tensor.matmul(
                                    wxps[:],
                                    wwT[:, e, jb * P:(jb + 1) * P],
                                    g_lo[:, e, ssl],
                                    start=(e == 0),
                                    stop=(e == NE - 1),
                                )
                            t0 = c * GT + s * (CHUNK // BL)
                            nc.scalar.activation(
                                wx_view[:, t0:t0 + CHUNK // BL, jb, :],
                                wxps[:],
                                mybir.ActivationFunctionType.Identity,
                                bias=bias[:, jb:jb + 1],
                                scale=1.0,
                            )

            # --- scan: 512 steps ---------------------------------------
            # Two PSUM tiles [128, 32] per step (j-blocks {0,1} / {2,3}).
            # wx is injected through the PE: an identity-weight matmul with
            # start=True opens each accumulation group with wx_t already in
            # PSUM (sets has_written), so the drain is a single ACT tanh
            # reading PSUM into an fp16 h tile.  The identity matmuls depend
            # only on static data, filling the PE bubble while it waits for
            # the previous step's tanh.
            hT32 = cpool.tile([P, NJ * BL], F32)
            wx_half = wx[:].rearrange(
                "p (t half c) -> p t half c", t=T, half=2, c=2 * BL
            )
            with tc.tile_pool(name="scan_ps", bufs=4, space="PSUM") as scps:
                h_cur = []
                for kp in range(NK // 2):
                    h0 = spool.tile([P, 2 * BL], F16, tag="h")
                    nc.gpsimd.memset(h0[:], 0.0)
                    h_cur.append(h0)
                for t in range(T):
                    pss = []
                    for half in range(2):
                        ps = scps.tile([P, 2 * BL], F32, tag="ps")
                        nc.tensor.matmul(
                            ps[:], ident[:], wx_half[:, t, half, :],
                            start=True, stop=False,
                        )
                        pss.append(ps)
                    h_nxt = []
                    for half in range(2):
                        ps = pss[half]
                        for jj in range(2):
                            jb = half * 2 + jj
                            for k in range(NK):
                                nc.tensor.matmul(
                                    ps[:, jj * BL:(jj + 1) * BL],
                                    uwT[:, k, jb * P:(jb + 1) * P],
                                    h_cur[k // 2][:, (k % 2) * BL:(k % 2 + 1) * BL],
                                    start=False,
                                    stop=(jj == 1 and k == NK - 1),
                                )
                        if t < T - 1:
                            h = spool.tile([P, 2 * BL], F16, tag="h")
                            nc.scalar.activation(
                                h[:], ps[:], mybir.ActivationFunctionType.Tanh
                            )
                            h_nxt.append(h)
                        else:
                            nc.scalar.activation(
                                hT32[:, half * 2 * BL:(half + 1) * 2 * BL],
                                ps[:],
                                mybir.ActivationFunctionType.Tanh,
                            )
                    h_cur = h_nxt

            nc.sync.dma_start(out_d[:], hT32[:])

    nc.compile()
    return nc


def _get_module():
    if "nc" not in _CACHED:
        _CACHED["nc"] = build_module()
    return _CACHED["nc"]


def _host_inputs(x, emb, Ww, Wb, Uw, Ub):
    emb16 = emb.astype(np.float16)
    zrow = np.zeros((1, EMB), np.float16)
    emb_lo16 = np.ascontiguousarray(np.vstack([zrow, emb16[:VLO]]))
    emb_hi16 = np.ascontiguousarray(np.vstack([zrow, emb16[VLO:]]))
    wwT16 = np.ascontiguousarray(
        Ww.T.reshape(NE, P, HID).transpose(1, 0, 2)
    ).astype(np.float16)
    uwT16 = np.ascontiguousarray(
        Uw.T.reshape(NK, P, HID).transpose(1, 0, 2)
    ).astype(np.float16)
    bias32 = np.ascontiguousarray(
        (Wb + Ub).astype(np.float32).reshape(NJ, P).T
    )

    def wrap16(idx):
        # index i lives at [i % 16, i // 16]; the 16-row wrap must be
        # replicated to all 8 GpSimd core groups (partitions 16q..16q+15)
        return np.ascontiguousarray(
            np.tile(idx.reshape(NTOK // 16, 16).T, (8, 1))
        )

    in_maps = []
    for c in range(NCORES):
        xc = x[c * BL:(c + 1) * BL, :]              # [16, 512] int
        tok = np.ascontiguousarray(xc.T).reshape(-1).astype(np.int64)
        lo = np.where(tok < VLO, tok + 1, 0).astype(np.int16)
        hi = np.where(tok >= VLO, tok - VLO + 1, 0).astype(np.int16)
        in_maps.append({
            "emb_lo16": emb_lo16,
            "emb_hi16": emb_hi16,
            "wwT16": wwT16,
            "uwT16": uwT16,
            "bias32": bias32,
            "idx_lo16": wrap16(lo),
            "idx_hi16": wrap16(hi),
        })
    return in_maps


def _ht_to_h(ht):
    # ht [128, 64] f32, ht[p, kb*16+b] = h[b, kb*128+p]
    return np.ascontiguousarray(
        ht.reshape(P, NJ, BL).transpose(2, 1, 0).reshape(BL, HID)
    )


def run_device(x, emb, Ww, Wb, Uw, Ub, trace=False):
    from concourse.bass_utils import run_bass_kernel_spmd

    nc = _get_module()
    in_maps = _host_inputs(x, emb, Ww, Wb, Uw, Ub)
    res = run_bass_kernel_spmd(
        nc, in_maps, list(range(NCORES)), trace=trace,
        trace_cores=list(range(NCORES)) if trace else None,
    )
    hs = [_ht_to_h(res.results[c]["ht_out"]) for c in range(NCORES)]
    h_full = np.concatenate(hs, axis=0)              # [128, 512] f32
    return h_full, res


def kernel(x, emb, Ww, Wb, Uw, Ub, Vw, Vb):
    x = np.asarray(x)
    emb = np.asarray(emb, dtype=np.float32)
    Ww = np.asarray(Ww, dtype=np.float32)
    Wb = np.asarray(Wb, dtype=np.float32)
    Uw = np.asarray(Uw, dtype=np.float32)
    Ub = np.asarray(Ub, dtype=np.float32)
    Vw = np.asarray(Vw, dtype=np.float32)
    Vb = np.asarray(Vb, dtype=np.float32)

    h_full, _ = run_device(x, emb, Ww, Wb, Uw, Ub, trace=False)
    logits = h_full @ Vw.T.astype(np.float32) + Vb   # [128, 2]
    return logits.astype(np.float32)


# revision 19
# speedup vs baseline: 1.1621x; 1.1621x over previous
"""Trainium2 Bass kernel for the Elman-RNN reference (nn_Baseline_78005196030499).

Architecture (per core, data-parallel over batch, B=128 -> 16 rows/core):
  1. Host: cast emb to fp16, pre-transpose weights, build t-major int32 token
     index tables.
  2. Device prep: indirect-DMA gather of embedding rows (fp16), PE transpose
     to put E on partitions, fp16 input-projection GEMM (fp32 PSUM) with
     Wb+Ub folded in via the ACT epilogue -> wx [128, T*64] fp32 resident in
     SBUF (layout col = t*64 + jb*16 + b).
  3. Device scan: 512 sequential steps; per step 16 fp16 matmuls
     (stationary UwT tiles [128,128], moving hT [128,16]) accumulating into
     4 PSUM banks (one per output j-block), drained by DVE add (psum + wx_t)
     and ACT tanh back to fp16 hT tiles.  j-major order lets the drains trail
     the PE stream so PE never stalls.
  4. Final hidden state written out in fp32; the tiny V-projection and the
     batch gather happen on host in fp32.
"""
import sys

sys.path.insert(0, "/opt/trn_rl_repo")

import numpy as np

import concourse.bass as bass
import concourse.tile as tile
from concourse import bacc, mybir
from concourse.masks import make_identity

# Problem shape (hardcoded per contract).
VOCAB, EMB, HID, OUT = 50257, 256, 512, 2
B, T = 128, 512
NCORES = 8
BL = B // NCORES          # batch rows per core = 16
NTOK = BL * T             # tokens per core = 8192
P = 128
NJ = HID // P             # 4 j-blocks
NK = HID // P             # 4 k-chunks
NE = EMB // P             # 2 e-chunks
CHUNK = 512               # tokens per GEMM sub-block
GCHUNK = 512              # tokens per gather chunk (HW limit: ~512 idxs/gather)
NGC = NTOK // GCHUNK      # 4
GT = GCHUNK // BL         # timesteps per gather chunk = 128
VLO = 32767               # tokens < VLO come from emb_lo (row tok+1)
NLO = VLO + 1             # emb_lo rows (incl. leading zero row)
NHI = VOCAB - VLO + 1     # emb_hi rows (incl. leading zero row)

F16 = mybir.dt.float16
F32 = mybir.dt.float32
I16 = mybir.dt.int16

_CACHED = {}


def build_module():
    nc = bacc.Bacc("TRN2", target_bir_lowering=False, debug=False)

    emb_lo_d = nc.dram_tensor("emb_lo16", [NLO, EMB], F16, kind="ExternalInput")
    emb_hi_d = nc.dram_tensor("emb_hi16", [NHI, EMB], F16, kind="ExternalInput")
    wwT_d = nc.dram_tensor("wwT16", [P, NE, HID], F16, kind="ExternalInput")
    uwT_d = nc.dram_tensor("uwT16", [P, NK, HID], F16, kind="ExternalInput")
    bias_d = nc.dram_tensor("bias32", [P, NJ], F32, kind="ExternalInput")
    idxlo_d = nc.dram_tensor("idx_lo16", [P, NTOK // 16], I16, kind="ExternalInput")
    idxhi_d = nc.dram_tensor("idx_hi16", [P, NTOK // 16], I16, kind="ExternalInput")
    out_d = nc.dram_tensor("ht_out", [P, NJ * BL], F32, kind="ExternalOutput")

    with tile.TileContext(nc) as tc:
        with (
            tc.tile_pool(name="const", bufs=1) as cpool,
            tc.tile_pool(name="wxpool", bufs=1) as wxpool,
            tc.tile_pool(name="gather", bufs=2) as gpool,
            tc.tile_pool(name="scan_sb", bufs=8) as spool,
            tc.tile_pool(name="prep_ps", bufs=2, space="PSUM") as ppool,
            tc.tile_pool(name="scan_ps", bufs=4, space="PSUM") as scps,
        ):
            # --- resident constants -------------------------------------
            uwT = cpool.tile([P, NK, HID], F16)
            nc.sync.dma_start(uwT[:], uwT_d[:])
            wwT = cpool.tile([P, NE, HID], F16)
            nc.sync.dma_start(wwT[:], wwT_d[:])
            bias = cpool.tile([P, NJ], F32)
            nc.sync.dma_start(bias[:], bias_d[:])
            idxlo = cpool.tile([P, NTOK // 16], I16)
            nc.sync.dma_start(idxlo[:], idxlo_d[:])
            idxhi = cpool.tile([P, NTOK // 16], I16)
            nc.sync.dma_start(idxhi[:], idxhi_d[:])
            ident = cpool.tile([P, P], F16)
            make_identity(nc, ident[:])

            # per-chunk wx tiles: [128, GT * 64] fp16, col = t*64 + jb*16 + b
            wx_tiles = [
                wxpool.tile([P, GT * NJ * BL], F16, name=f"wx{c}")
                for c in range(NGC)
            ]
            wx_views = [
                w[:].rearrange("p (t j b) -> p t j b", t=GT, j=NJ, b=BL)
                for w in wx_tiles
            ]
            wx_halves = [
                w[:].rearrange("p (t half c) -> p t half c", t=GT, half=2, c=2 * BL)
                for w in wx_tiles
            ]

            # --- prep, expressed as a list of schedulable items ---------
            # dma_gather(transpose=True) lands embedding rows directly in
            # the [e-partition, e-chunk, token] layout the GEMM wants.  The
            # vocab is split across two fp16 tables (int16 index limit),
            # each with a leading all-zero row; out-of-range tokens hit the
            # zero row, so the two gathers simply sum.  Epilogue bias-add
            # runs on the (otherwise idle) DVE.  Items for later chunks are
            # sprinkled between scan steps: gathers run on the idle Q7 and
            # GEMM matmuls soak up the PE bubbles while it waits for tanh.
            def chunk_items(c):
                st = {}
                isl = slice(c * (GCHUNK // 16), (c + 1) * (GCHUNK // 16))

                def it_gather_lo():
                    st["g"] = gpool.tile([P, NE, GCHUNK], F16, name="g_lo", tag="glo")
                    nc.gpsimd.dma_gather(
                        out_ap=st["g"][:], in_ap=emb_lo_d[:],
                        idxs_ap=idxlo[:, isl],
                        num_idxs=GCHUNK, num_idxs_reg=GCHUNK,
                        elem_size=EMB, transpose=True,
                    )

                def it_gather_hi():
                    st["g2"] = gpool.tile([P, NE, GCHUNK], F16, name="g_hi", tag="ghi")
                    nc.gpsimd.dma_gather(
                        out_ap=st["g2"][:], in_ap=emb_hi_d[:],
                        idxs_ap=idxhi[:, isl],
                        num_idxs=GCHUNK, num_idxs_reg=GCHUNK,
                        elem_size=EMB, transpose=True,
                    )

                def it_merge():
                    nc.vector.tensor_add(st["g"][:], st["g"][:], st["g2"][:])

                def mk_gemm(jb):
                    def it_gemm():
                        ps = ppool.tile([P, GCHUNK], F32, name="wxps", tag="wxps")
                        for e in range(NE):
                            nc.tensor.matmul(
                                ps[:],
                                wwT[:, e, jb * P:(jb + 1) * P],
                                st["g"][:, e, :],
                                start=(e == 0),
                                stop=(e == NE - 1),
                            )
                        st[jb] = ps
                    return it_gemm

                def mk_epi(jb):
                    def it_epi():
                        nc.vector.tensor_scalar_add(
                            wx_views[c][:, :, jb, :], st[jb][:],
                            bias[:, jb:jb + 1],
                        )
                    return it_epi

                out = [it_gather_lo, it_gather_hi, it_merge]
                for jb in range(NJ):
                    out.append(mk_gemm(jb))
                    out.append(mk_epi(jb))
                return out

            UPFRONT = min(2, NGC)
            for c in range(UPFRONT):
                for it in chunk_items(c):
                    it()
            deferred = []
            for c in range(UPFRONT, NGC):
                deferred.extend(chunk_items(c))
            # spread deferred items over the first ~82% of the scan
            sched = {}
            if deferred:
                span = int(T * 0.82)
                for i, it in enumerate(deferred):
                    step = int(i * span / len(deferred))
                    sched.setdefault(step, []).append(it)

            # --- scan: 512 steps ---------------------------------------
            # Two PSUM tiles [128, 32] per step (j-blocks {0,1} / {2,3}).
            # wx is injected through the PE: an identity-weight matmul with
            # start=True opens each accumulation group with wx_t already in
            # PSUM (sets has_written), so the drain is a single ACT tanh
            # reading PSUM into an fp16 h tile.  The identity matmuls depend
            # only on static data, filling the PE bubble while it waits for
            # the previous step's tanh.
            hT32 = cpool.tile([P, NJ * BL], F32)
            h_cur = []
            for kp in range(NK // 2):
                h0 = spool.tile([P, 2 * BL], F16, tag="h")
                nc.gpsimd.memset(h0[:], 0.0)
                h_cur.append(h0)
            for t in range(T):
                for it in sched.get(t, ()):
                    it()
                wxh = wx_halves[t // GT]
                tl = t % GT
                pss = []
                for half in range(2):
                    ps = scps.tile([P, 2 * BL], F32, tag="ps")
                    nc.tensor.matmul(
                        ps[:], ident[:], wxh[:, tl, half, :],
                        start=True, stop=False,
                    )
                    pss.append(ps)
                h_nxt = []
                for half in range(2):
                    ps = pss[half]
                    for jj in range(2):
                        jb = half * 2 + jj
                        for k in range(NK):
                            nc.tensor.matmul(
                                ps[:, jj * BL:(jj + 1) * BL],
                                uwT[:, k, jb * P:(jb + 1) * P],
                                h_cur[k // 2][:, (k % 2) * BL:(k % 2 + 1) * BL],
                                start=False,
                                stop=(jj == 1 and k == NK - 1),
                            )
                    if t < T - 1:
                        h = spool.tile([P, 2 * BL], F16, tag="h")
                        nc.scalar.activation(
                            h[:], ps[:], mybir.ActivationFunctionType.Tanh
                        )
                        h_nxt.append(h)
                    else:
                        nc.scalar.activation(
                            hT32[:, half * 2 * BL:(half + 1) * 2 * BL],
                            ps[:],
                            mybir.ActivationFunctionType.Tanh,
                        )
                h_cur = h_nxt

            nc.sync.dma_start(out_d[:], hT32[:])

    nc.compile()
    return nc


def _get_module():
    if "nc" not in _CACHED:
        _CACHED["nc"] = build_module()
    return _CACHED["nc"]


def _host_inputs(x, emb, Ww, Wb, Uw, Ub):
    emb16 = emb.astype(np.float16)
    zrow = np.zeros((1, EMB), np.float16)
    emb_lo16 = np.ascontiguousarray(np.vstack([zrow, emb16[:VLO]]))
    emb_hi16 = np.ascontiguousarray(np.vstack([zrow, emb16[VLO:]]))
    wwT16 = np.ascontiguousarray(
        Ww.T.reshape(NE, P, HID).transpose(1, 0, 2)
    ).astype(np.float16)
    uwT16 = np.ascontiguousarray(
        Uw.T.reshape(NK, P, HID).transpose(1, 0, 2)
    ).astype(np.float16)
    bias32 = np.ascontiguousarray(
        (Wb + Ub).astype(np.float32).reshape(NJ, P).T
    )

    def wrap16(idx):
        # index i lives at [i % 16, i // 16]; the 16-row wrap must be
        # replicated to all 8 GpSimd core groups (partitions 16q..16q+15)
        return np.ascontiguousarray(
            np.tile(idx.reshape(NTOK // 16, 16).T, (8, 1))
        )

    in_maps = []
    for c in range(NCORES):
        xc = x[c * BL:(c + 1) * BL, :]              # [16, 512] int
        tok = np.ascontiguousarray(xc.T).reshape(-1).astype(np.int64)
        lo = np.where(tok < VLO, tok + 1, 0).astype(np.int16)
        hi = np.where(tok >= VLO, tok - VLO + 1, 0).astype(np.int16)
        in_maps.append({
            "emb_lo16": emb_lo16,
            "emb_hi16": emb_hi16,
            "wwT16": wwT16,
            "uwT16": uwT16,
            "bias32": bias32,
            "idx_lo16": wrap16(lo),
            "idx_hi16": wrap16(hi),
        })
    return in_maps


def _ht_to_h(ht):
    # ht [128, 64] f32, ht[p, kb*16+b] = h[b, kb*128+p]
    return np.ascontiguousarray(
        ht.reshape(P, NJ, BL).transpose(2, 1, 0).reshape(BL, HID)
    )


def run_device(x, emb, Ww, Wb, Uw, Ub, trace=False):
    from concourse.bass_utils import run_bass_kernel_spmd

    nc = _get_module()
    in_maps = _host_inputs(x, emb, Ww, Wb, Uw, Ub)
    res = run_bass_kernel_spmd(
        nc, in_maps, list(range(NCORES)), trace=trace,
        trace_cores=list(range(NCORES)) if trace else None,
    )
    hs = [_ht_to_h(res.results[c]["ht_out"]) for c in range(NCORES)]
    h_full = np.concatenate(hs, axis=0)              # [128, 512] f32
    return h_full, res


def kernel(x, emb, Ww, Wb, Uw, Ub, Vw, Vb):
    x = np.asarray(x)
    emb = np.asarray(emb, dtype=np.float32)
    Ww = np.asarray(Ww, dtype=np.float32)
    Wb = np.asarray(Wb, dtype=np.float32)
    Uw = np.asarray(Uw, dtype=np.float32)
    Ub = np.asarray(Ub, dtype=np.float32)
    Vw = np.asarray(Vw, dtype=np.float32)
    Vb = np.asarray(Vb, dtype=np.float32)

    h_full, _ = run_device(x, emb, Ww, Wb, Uw, Ub, trace=False)
    logits = h_full @ Vw.T.astype(np.float32) + Vb   # [128, 2]
    return logits.astype(np.float32)
